# revision 45
# baseline (speedup 1.0000x reference)
"""Distributed Trainium2 (Bass) kernel for nn_AnchorLoss — rank-R feature path.

Reference:
  pos  = embedding + abs_coords                     [B, N, D],  B=8, N=2048, D=2
  K_ij = exp(-||pos_i - pos_j||^2 / T)
  loss = sum over (b,i,j) with patch_mask==1 of (1 - K_ij)

Math: the Gaussian kernel over ~N(0,2) 2-D data is smooth, so it admits a
low-rank Mercer/Taylor expansion
  K(u,v) = e^{-r_u/T} e^{-r_v/T} e^{u.v/5}
         ~= sum_f Phi_f(u) Phi_f(v),
  Phi_{k1,k2}(u) = e^{-r_u/T} (x/sqrt5)^{k1} (y/sqrt5)^{k2} / sqrt(k1! k2!)
truncated at total degree KDEG=6 (R=28 features, padded to RP=32; measured
end-to-end rel err ~7e-5, gate is 2e-2). With M~ = upper-tri((mask+mask^T)/2,
diag=0):
  loss = count1 - diag_cnt - 2*S,   S = trace(Phi^T M~ Phi)
so the whole masked pairwise sum becomes TensorE matmuls — ZERO on-chip exp
(the baseline's ScalarE exp stream was the measured bottleneck at ~21us).

Distribution: batch b -> NeuronCore b (8 cores). Host combines scalars.

Kernel (per core):
  Psi^T[f, i] = sum_j Phi8[j, f] * Mt[j, i]  accumulated in PSUM f32 over the
  16 column-blocks of the triangle Mt = M~^T (block J = rows j in
  [128J, 128J+128) x cols i in [0, 128(J+1)), fp8 e4m3 — mask values
  {0, 0.5, 1} are fp8-exact; Phi is fp8 with power-of-2 per-feature scales,
  un-scaled on the fp16 PhiT side). Adjacent blocks are PAIRED (15,14)...
  (1,0) sharing one region; each 512-col chunk runs as two plain fp8
  matmuls (sub0/sub1) plus a 128-col strip for the odd block. (A dual-fp8
  DoubleRow variant exists behind USE_DR but is disabled: it intermittently
  crashes the exec unit and buys nothing — the stream is DMA-bound.)
  PSUM start_tensor_calc marks the whole 2KB bank pending-zero, so ONLY the
  bank-aligned first-touch matmul of each bank sets start=True; later
  first-writes inherit the pending flag and overwrite, accumulators add.
  Pairs run DESCENDING so high PSUM cols finalize first; the DVE folds each
  finalized span against PhiT via scalar_tensor_tensor (one op: mult +
  row-sum accumulate). A burst of NWARM dummy matmuls at block start keeps
  the PE busy so the HAM clock gate lifts 1.2->2.4 GHz before real work.
  DMA: the 2.2MB fp8 triangle streams as 12 units in strict consumption
  order, alternating between the sync and scalar HWDGE queues (fair
  round-robin draining otherwise delays the head block to the very end);
  within a pair region the two sub-blocks are interleaved at 512-col chunk
  granularity so each chunk's operands arrive together.
"""

from contextlib import ExitStack

import math
import numpy as np
from ml_dtypes import float8_e4m3

B, N, D = 8, 2048, 2
TEMPERATURE = 10.0
P = 128
KDEG = 6
R = (KDEG + 1) * (KDEG + 2) // 2          # 28 real features
RP = 32                                   # padded (DoubleRow needs 16B-aligned steps)
NPAIR = 8                                 # pairs (15,14) ... (1,0)
CHUNK = 512                               # PSUM bank width in f32
NWARM = 40                                # dummy matmuls to un-throttle the PE HAM early
DR_PAIRS = set()                            # pairs using DoubleRow
USE_DR = False                            # DoubleRow dual-fp8 perf mode
WARM_FD = 128                             # fat enough that the HAM sees real PE activity

TRACE = False        # set True (see test.py) to neuron-profile the run
LAST_RESULTS = None  # BassKernelResults of the last run when TRACE

_cache = {}


def _pair_W(m):
    """Shared (paired) column count of pair m = blocks (2m+1, 2m)."""
    return (2 * m + 1) * P


def _chunks_of_pair(m):
    """Chunk list over the paired span [0, W): 512-grid, but the final 128
    cols [W-128, W) form their own chunk — they are the only paired cols
    whose LAST writer is this pair, and stop_tensor_calc (which gates PSUM
    read visibility on hardware) must be exact per element."""
    W = _pair_W(m)
    bounds = list(range(0, W - P, CHUNK)) + [W - P, W]
    return list(zip(bounds[:-1], bounds[1:]))


# region layout (consumption order, pairs descending m=7..0):
#   pair m region = [sub0 c | sub1 c]-interleaved 512-chunks + 128-col strip
_OFFP = {}
_off = 0
for _m in range(NPAIR - 1, -1, -1):
    _OFFP[_m] = _off
    _off += 2 * _pair_W(_m) + P
MTW = _off                                # 17408

# DMA units: ONE per pair, in consumption order. The DMA head is
# descriptor-rate bound (~140-400ns/descriptor for the first ~400
# descriptors regardless of size), so wide units (128 descriptors of up to
# 4KB each) move ~4x the bytes through the slow head window and leave no
# mid-stream stalls.
UNITS = []
for _m in range(NPAIR - 1, -1, -1):
    _base, _end = _OFFP[_m], _OFFP[_m] + 2 * _pair_W(_m) + P
    if _m == NPAIR - 1:
        # halve only the head pair: the first matmul gates on 254KB not 508KB
        UNITS.append((_base, _base + 2048))
        UNITS.append((_base + 2048, _end))
    else:
        UNITS.append((_base, _end))
SYNC_IDX = list(range(0, len(UNITS), 2))
SCAL_IDX = list(range(1, len(UNITS), 2))

# DVE fold spans (c0, c1) -> last pair touching span is m = c0 // 256
DVE_SPANS = [(1536, 2048), (1024, 1536), (512, 1024), (256, 512), (0, 256)]


def _build():
    from concourse import bacc, mybir

    nc = bacc.Bacc(enable_partition_id=False)
    f32 = mybir.dt.float32
    f16 = mybir.dt.float16
    f8 = mybir.dt.float8e4

    phist_d = nc.declare_dram_parameter("phist", [P, NPAIR * 2 * RP], f8, isOutput=False)
    phit_d = nc.declare_dram_parameter("phit", [RP, N], f16, isOutput=False)
    mt_d = nc.declare_dram_parameter("mt", [P, MTW], f8, isOutput=False)
    out_d = nc.declare_dram_parameter("out", [RP, len(DVE_SPANS)], f32, isOutput=True)

    def unit_for(rc0):
        for gi, (u0, u1) in enumerate(UNITS):
            if u0 <= rc0 < u1:
                return gi
        raise AssertionError(rc0)

    # pe_sem value after the last chunk of pair m (pairs run descending)
    done_after = {}
    cnt = 0
    for m in range(NPAIR - 1, -1, -1):
        cnt += len(_chunks_of_pair(m)) + 1        # + strip
        done_after[m] = cnt
    dve_thr = [done_after[c0 // 256] for (c0, c1) in DVE_SPANS]

    with ExitStack() as ctx:
        phist = ctx.enter_context(nc.sbuf_tensor("phist_sb", [P, NPAIR * 2 * RP], f8))
        phit = ctx.enter_context(nc.sbuf_tensor("phit_sb", [RP, N], f16))
        mt = ctx.enter_context(nc.sbuf_tensor("mt_sb", [P, MTW], f8))
        acc = ctx.enter_context(nc.sbuf_tensor("acc", [RP, len(DVE_SPANS)], f32))
        prod = ctx.enter_context(nc.sbuf_tensor("prod", [RP, CHUNK], f32))
        dum_w = ctx.enter_context(nc.sbuf_tensor("dum_w", [P, 4], f16))
        dum_x = ctx.enter_context(nc.sbuf_tensor("dum_x", [P, WARM_FD], f8))
        ps = ctx.enter_context(nc.psum_tensor("ps", [P, N], f32))
        ps_warm = ctx.enter_context(nc.psum_tensor("ps_warm", [P, WARM_FD], f32))

        u_sems = [ctx.enter_context(nc.semaphore(f"u{g}")) for g in range(len(UNITS))]
        st_sem = ctx.enter_context(nc.semaphore("st"))
        pt_sem = ctx.enter_context(nc.semaphore("pt"))
        pe_sem = ctx.enter_context(nc.semaphore("pe"))
        dve_sem = ctx.enter_context(nc.semaphore("dve"))
        odma_sem = ctx.enter_context(nc.semaphore("odma"))
        odma2_sem = ctx.enter_context(nc.semaphore("odma2"))
        block = ctx.enter_context(nc.Block(no_gpsimd_drain=True))

        @block.sync
        def _(sync):
            sync.dma_start(out=phist[:, :], in_=phist_d[:, :]).then_inc(st_sem, 16)
            for gi in SYNC_IDX:
                c0, c1 = UNITS[gi]
                sync.dma_start(
                    out=mt[0:P, c0:c1], in_=mt_d[0:P, c0:c1]
                ).then_inc(u_sems[gi], 16)
            # output slots 0-3 as soon as their folds are done; the gen and
            # completion latency overlap the final fold
            nsp = len(DVE_SPANS)
            sync.wait_ge(dve_sem, nsp - 1)
            sync.dma_start(out=out_d[:, 0:nsp - 1], in_=acc[:, 0:nsp - 1]).then_inc(odma_sem, 16)
            sync.wait_ge(odma_sem, 16)

        @block.scalar
        def _(scalar):
            for gi in SCAL_IDX:
                c0, c1 = UNITS[gi]
                scalar.dma_start(
                    out=mt[0:P, c0:c1], in_=mt_d[0:P, c0:c1]
                ).then_inc(u_sems[gi], 16)
                if gi == 5:
                    # phit needed by the first DVE fold (~1/3 into the stream)
                    scalar.dma_start(out=phit[:, :], in_=phit_d[:, :]).then_inc(pt_sem, 16)
            # final fold's slot from this queue, in parallel with sync's
            nsp = len(DVE_SPANS)
            scalar.wait_ge(dve_sem, nsp)
            with nc.allow_non_contiguous_dma(reason="32x4B final slot"):
                scalar.dma_start(out=out_d[:, nsp - 1:nsp], in_=acc[:, nsp - 1:nsp]).then_inc(odma2_sem, 16)
            scalar.wait_ge(odma2_sem, 16)

        @block.tensor
        def _(tensor):
            # HAM warmup: garbage matmuls into a scratch bank, no data deps
            for _w in range(NWARM):
                tensor.matmul(
                    ps_warm[0:4, 0:WARM_FD], lhsT=dum_w[:, :], rhs=dum_x[:, :],
                    start=True, stop=True,
                )
            tensor.wait_ge(st_sem, 16)
            waited = set()
            for m in range(NPAIR - 1, -1, -1):
                base = _OFFP[m]
                W = _pair_W(m)
                lhs2 = phist[0:P, m * 2 * RP:(m + 1) * 2 * RP].rearrange(
                    "k (two r) -> k two r", two=2)
                for (c0, c1) in _chunks_of_pair(m):
                    gi = unit_for(base + 2 * c0)
                    if gi not in waited:
                        waited.add(gi)
                        tensor.wait_ge(u_sems[gi], 16)
                    w = c1 - c0
                    if USE_DR and m in DR_PAIRS:
                        rhs2 = mt[0:P, base + 2 * c0:base + 2 * c1].rearrange(
                            "k (two w) -> k two w", two=2)
                        tensor.matmul(
                            ps[0:RP, c0:c1], lhsT=lhs2, rhs=rhs2,
                            start=(m == NPAIR - 1 and c0 % CHUNK == 0),
                            stop=(c0 // 256 == m),
                            perf_mode=mybir.MatmulPerfMode.DoubleRow,
                        ).then_inc(pe_sem, 1)
                    else:
                        tensor.matmul(
                            ps[0:RP, c0:c1],
                            lhsT=phist[0:P, m * 2 * RP:m * 2 * RP + RP],
                            rhs=mt[0:P, base + 2 * c0:base + 2 * c0 + w],
                            start=(m == NPAIR - 1 and c0 % CHUNK == 0), stop=False,
                        )
                        tensor.matmul(
                            ps[0:RP, c0:c1],
                            lhsT=phist[0:P, m * 2 * RP + RP:(m + 1) * 2 * RP],
                            rhs=mt[0:P, base + 2 * c0 + w:base + 2 * c1],
                            start=False,
                            stop=(c0 // 256 == m),
                        ).then_inc(pe_sem, 1)
                # strip: odd block's last 128 cols, plain fp8 matmul
                gi = unit_for(base + 2 * W)
                if gi not in waited:
                    waited.add(gi)
                    tensor.wait_ge(u_sems[gi], 16)
                tensor.matmul(
                    ps[0:RP, W:W + P],
                    lhsT=phist[0:P, m * 2 * RP:m * 2 * RP + RP],
                    rhs=mt[0:P, base + 2 * W:base + 2 * W + P],
                    start=(m == NPAIR - 1 and W % CHUNK == 0),
                    stop=True,
                ).then_inc(pe_sem, 1)

        @block.vector
        def _(vector):
            vector.wait_ge(pt_sem, 16)
            for si, (c0, c1) in enumerate(DVE_SPANS):
                vector.wait_ge(pe_sem, dve_thr[si])
                w = c1 - c0
                # single-pass fold: out=(ps*1.0)*phit, accum_out=row-sum
                # (tensor_tensor_reduce crashes the exec unit on this runtime)
                vector.scalar_tensor_tensor(
                    out=prod[0:RP, 0:w],
                    in0=ps[0:RP, c0:c1],
                    scalar=1.0,
                    in1=phit[0:RP, c0:c1],
                    op0=mybir.AluOpType.mult,
                    op1=mybir.AluOpType.mult,
                    accum_out=acc[0:RP, si:si + 1],
                ).then_inc(dve_sem, 1)

    nc.compile()
    return nc


_FEATS = [(k1, k2) for k1 in range(KDEG + 1) for k2 in range(KDEG + 1 - k1)]


def _features(pos):
    """pos [N, 2] f64 -> Phi [N, R] f64."""
    x, y = pos[:, 0], pos[:, 1]
    base = np.exp(-(x * x + y * y) / TEMPERATURE)
    cols = [
        base * (x / math.sqrt(5.0)) ** k1 * (y / math.sqrt(5.0)) ** k2
        / math.sqrt(math.factorial(k1) * math.factorial(k2))
        for (k1, k2) in _FEATS
    ]
    return np.stack(cols, axis=1)


def _host_prep(embedding, abs_coords, patch_mask):
    in_maps = []
    count1 = 0
    diag_cnt = 0
    for b in range(B):
        pos = embedding[b].astype(np.float64) + abs_coords[b].astype(np.float64)
        Phi = _features(pos)                                        # [N, R]
        alpha = np.exp2(np.round(np.log2(160.0 / np.abs(Phi).max(axis=0))))
        Phi8 = np.clip(Phi * alpha, -240, 240).astype(float8_e4m3)  # [N, R]
        PhiT = (Phi / alpha).T.astype(np.float16)                   # [R, N]

        phist = np.zeros((P, NPAIR * 2 * RP), dtype=float8_e4m3)
        for m in range(NPAIR):
            J1, J0 = 2 * m + 1, 2 * m
            phist[:, m * 2 * RP:m * 2 * RP + R] = Phi8[J1 * P:(J1 + 1) * P, :]
            phist[:, m * 2 * RP + RP:m * 2 * RP + RP + R] = Phi8[J0 * P:(J0 + 1) * P, :]
        phit = np.zeros((RP, N), dtype=np.float16)
        phit[0:R, :] = PhiT

        m_ = patch_mask[b] == 1
        count1 += int(m_.sum())
        diag_cnt += int(np.trace(m_))
        msum = m_.astype(np.int8) + m_.astype(np.int8).T
        Mt8 = (np.triu(msum, k=1).astype(np.float32) * 0.5).astype(float8_e4m3)
        mt = np.zeros((P, MTW), dtype=float8_e4m3)
        for m in range(NPAIR):
            base = _OFFP[m]
            W = _pair_W(m)
            J1, J0 = 2 * m + 1, 2 * m
            for (c0, c1) in _chunks_of_pair(m):
                w = c1 - c0
                mt[:, base + 2 * c0:base + 2 * c0 + w] = \
                    Mt8[c0:c1, J1 * P:(J1 + 1) * P].T
                mt[:, base + 2 * c0 + w:base + 2 * c1] = \
                    Mt8[c0:c1, J0 * P:(J0 + 1) * P].T
            mt[:, base + 2 * W:base + 2 * W + P] = \
                Mt8[W:W + P, J1 * P:(J1 + 1) * P].T
        in_maps.append({"phist": phist, "phit": phit, "mt": mt})
    return in_maps, count1, diag_cnt


def kernel(embedding, abs_coords, patch_mask):
    global LAST_RESULTS
    from concourse.bass_utils import run_bass_kernel_spmd

    embedding = np.asarray(embedding)
    abs_coords = np.asarray(abs_coords)
    patch_mask = np.asarray(patch_mask)

    if "nc" not in _cache:
        _cache["nc"] = _build()
    nc = _cache["nc"]

    in_maps, count1, diag_cnt = _host_prep(embedding, abs_coords, patch_mask)

    res = run_bass_kernel_spmd(
        nc, in_maps, core_ids=list(range(B)),
        trace=TRACE, trace_cores=[0] if TRACE else None,
    )
    LAST_RESULTS = res

    s_hw = sum(res.results[b]["out"].astype(np.float64).sum() for b in range(B))
    loss = np.float64(count1) - np.float64(diag_cnt) - 2.0 * s_hw
    return np.array(loss, dtype=np.float32)


# revision 46
# speedup vs baseline: 1.0841x; 1.0841x over previous
"""Distributed Trainium2 (Bass) kernel for nn_AnchorLoss — rank-R feature path.

Reference:
  pos  = embedding + abs_coords                     [B, N, D],  B=8, N=2048, D=2
  K_ij = exp(-||pos_i - pos_j||^2 / T)
  loss = sum over (b,i,j) with patch_mask==1 of (1 - K_ij)

Math: the Gaussian kernel over ~N(0,2) 2-D data is smooth, so it admits a
low-rank Mercer/Taylor expansion
  K(u,v) = e^{-r_u/T} e^{-r_v/T} e^{u.v/5}
         ~= sum_f Phi_f(u) Phi_f(v),
  Phi_{k1,k2}(u) = e^{-r_u/T} (x/sqrt5)^{k1} (y/sqrt5)^{k2} / sqrt(k1! k2!)
truncated at total degree KDEG=6 (R=28 features, padded to RP=32; measured
end-to-end rel err ~7e-5, gate is 2e-2). With M~ = upper-tri((mask+mask^T)/2,
diag=0):
  loss = count1 - diag_cnt - 2*S,   S = trace(Phi^T M~ Phi)
so the whole masked pairwise sum becomes TensorE matmuls — ZERO on-chip exp
(the baseline's ScalarE exp stream was the measured bottleneck at ~21us).

Distribution: batch b -> NeuronCore b (8 cores). Host combines scalars.

Kernel (per core):
  Psi^T[f, i] = sum_j Phi8[j, f] * Mt[j, i]  accumulated in PSUM f32 over the
  16 column-blocks of the triangle Mt = M~^T (block J = rows j in
  [128J, 128J+128) x cols i in [0, 128(J+1)), fp8 e4m3 — mask values
  {0, 0.5, 1} are fp8-exact; Phi is fp8 with power-of-2 per-feature scales,
  un-scaled on the fp16 PhiT side). Adjacent blocks are PAIRED (15,14)...
  (1,0) sharing one region; each 512-col chunk runs as two plain fp8
  matmuls (sub0/sub1) plus a 128-col strip for the odd block. (A dual-fp8
  DoubleRow variant exists behind USE_DR but is disabled: it intermittently
  crashes the exec unit and buys nothing — the stream is DMA-bound.)
  PSUM start_tensor_calc marks the whole 2KB bank pending-zero, so ONLY the
  bank-aligned first-touch matmul of each bank sets start=True; later
  first-writes inherit the pending flag and overwrite, accumulators add.
  Pairs run DESCENDING so high PSUM cols finalize first; the DVE folds each
  finalized span against PhiT via scalar_tensor_tensor (one op: mult +
  row-sum accumulate). A burst of NWARM dummy matmuls at block start keeps
  the PE busy so the HAM clock gate lifts 1.2->2.4 GHz before real work.
  DMA: the 2.2MB fp8 triangle streams as 12 units in strict consumption
  order, alternating between the sync and scalar HWDGE queues (fair
  round-robin draining otherwise delays the head block to the very end);
  within a pair region the two sub-blocks are interleaved at 512-col chunk
  granularity so each chunk's operands arrive together.
"""

from contextlib import ExitStack

import math
import numpy as np
from ml_dtypes import float8_e4m3

B, N, D = 8, 2048, 2
TEMPERATURE = 10.0
P = 128
KDEG = 6
R = (KDEG + 1) * (KDEG + 2) // 2          # 28 real features
RP = 32                                   # padded (DoubleRow needs 16B-aligned steps)
NPAIR = 8                                 # pairs (15,14) ... (1,0)
CHUNK = 512                               # PSUM bank width in f32
NWARM = 46                                # dummy matmuls to un-throttle the PE HAM early
DR_PAIRS = set()                            # pairs using DoubleRow
USE_DR = False                            # DoubleRow dual-fp8 perf mode
WARM_FD = 128                             # fat enough that the HAM sees real PE activity

TRACE = False        # set True (see test.py) to neuron-profile the run
LAST_RESULTS = None  # BassKernelResults of the last run when TRACE

_cache = {}


def _pair_W(m):
    """Shared (paired) column count of pair m = blocks (2m+1, 2m)."""
    return (2 * m + 1) * P


def _chunks_of_pair(m):
    """Chunk list over the paired span [0, W): 512-grid, but the final 128
    cols [W-128, W) form their own chunk — they are the only paired cols
    whose LAST writer is this pair, and stop_tensor_calc (which gates PSUM
    read visibility on hardware) must be exact per element."""
    W = _pair_W(m)
    bounds = list(range(0, W - P, CHUNK)) + [W - P, W]
    return list(zip(bounds[:-1], bounds[1:]))


# region layout (consumption order, pairs descending m=7..0):
#   pair m region = [sub0 c | sub1 c]-interleaved 512-chunks + 128-col strip
_OFFP = {}
_off = 0
for _m in range(NPAIR - 1, -1, -1):
    _OFFP[_m] = _off
    _off += 2 * _pair_W(_m) + P
MTW = _off                                # 17408

# DMA units: ONE per pair, in consumption order. The DMA head is
# descriptor-rate bound (~140-400ns/descriptor for the first ~400
# descriptors regardless of size), so wide units (128 descriptors of up to
# 4KB each) move ~4x the bytes through the slow head window and leave no
# mid-stream stalls.
UNITS = []
for _m in range(NPAIR - 1, -1, -1):
    UNITS.append((_OFFP[_m], _OFFP[_m] + 2 * _pair_W(_m) + P))
SYNC_IDX = list(range(0, len(UNITS), 2))
SCAL_IDX = list(range(1, len(UNITS), 2))

# DVE fold spans (c0, c1) -> last pair touching span is m = c0 // 256
DVE_SPANS = [(1536, 2048), (1024, 1536), (512, 1024), (256, 512), (0, 256)]


def _build():
    from concourse import bacc, mybir

    nc = bacc.Bacc(enable_partition_id=False)
    f32 = mybir.dt.float32
    f16 = mybir.dt.float16
    f8 = mybir.dt.float8e4

    phist_d = nc.declare_dram_parameter("phist", [P, NPAIR * 2 * RP], f8, isOutput=False)
    phit_d = nc.declare_dram_parameter("phit", [RP, N], f16, isOutput=False)
    mt_d = nc.declare_dram_parameter("mt", [P, MTW], f8, isOutput=False)
    out_d = nc.declare_dram_parameter("out", [RP, len(DVE_SPANS)], f32, isOutput=True)

    def unit_for(rc0):
        for gi, (u0, u1) in enumerate(UNITS):
            if u0 <= rc0 < u1:
                return gi
        raise AssertionError(rc0)

    # pe_sem value after the last chunk of pair m (pairs run descending)
    done_after = {}
    cnt = 0
    for m in range(NPAIR - 1, -1, -1):
        cnt += len(_chunks_of_pair(m)) + 1        # + strip
        done_after[m] = cnt
    dve_thr = [done_after[c0 // 256] for (c0, c1) in DVE_SPANS]

    with ExitStack() as ctx:
        phist = ctx.enter_context(nc.sbuf_tensor("phist_sb", [P, NPAIR * 2 * RP], f8))
        phit = ctx.enter_context(nc.sbuf_tensor("phit_sb", [RP, N], f16))
        mt = ctx.enter_context(nc.sbuf_tensor("mt_sb", [P, MTW], f8))
        acc = ctx.enter_context(nc.sbuf_tensor("acc", [RP, len(DVE_SPANS)], f32))
        prod = ctx.enter_context(nc.sbuf_tensor("prod", [RP, CHUNK], f32))
        dum_w = ctx.enter_context(nc.sbuf_tensor("dum_w", [P, 4], f16))
        dum_x = ctx.enter_context(nc.sbuf_tensor("dum_x", [P, WARM_FD], f8))
        ps = ctx.enter_context(nc.psum_tensor("ps", [P, N], f32))
        ps_warm = ctx.enter_context(nc.psum_tensor("ps_warm", [P, WARM_FD], f32))

        u_sems = [ctx.enter_context(nc.semaphore(f"u{g}")) for g in range(len(UNITS))]
        st_sem = ctx.enter_context(nc.semaphore("st"))
        pt_sem = ctx.enter_context(nc.semaphore("pt"))
        pe_sem = ctx.enter_context(nc.semaphore("pe"))
        dve_sem = ctx.enter_context(nc.semaphore("dve"))
        odma_sem = ctx.enter_context(nc.semaphore("odma"))
        odma2_sem = ctx.enter_context(nc.semaphore("odma2"))
        block = ctx.enter_context(nc.Block(no_gpsimd_drain=True))

        @block.sync
        def _(sync):
            sync.dma_start(out=phist[:, :], in_=phist_d[:, :]).then_inc(st_sem, 16)
            for gi in SYNC_IDX:
                c0, c1 = UNITS[gi]
                sync.dma_start(
                    out=mt[0:P, c0:c1], in_=mt_d[0:P, c0:c1]
                ).then_inc(u_sems[gi], 16)
            # output slots 0-3 as soon as their folds are done; the gen and
            # completion latency overlap the final fold
            nsp = len(DVE_SPANS)
            sync.wait_ge(dve_sem, nsp - 1)
            sync.dma_start(out=out_d[:, 0:nsp - 1], in_=acc[:, 0:nsp - 1]).then_inc(odma_sem, 16)
            sync.wait_ge(odma_sem, 16)

        @block.scalar
        def _(scalar):
            for gi in SCAL_IDX:
                c0, c1 = UNITS[gi]
                scalar.dma_start(
                    out=mt[0:P, c0:c1], in_=mt_d[0:P, c0:c1]
                ).then_inc(u_sems[gi], 16)
                if gi == 5:
                    # phit needed by the first DVE fold (~1/3 into the stream)
                    scalar.dma_start(out=phit[:, :], in_=phit_d[:, :]).then_inc(pt_sem, 16)
            # final fold's slot from this queue, in parallel with sync's
            nsp = len(DVE_SPANS)
            scalar.wait_ge(dve_sem, nsp)
            with nc.allow_non_contiguous_dma(reason="32x4B final slot"):
                scalar.dma_start(out=out_d[:, nsp - 1:nsp], in_=acc[:, nsp - 1:nsp]).then_inc(odma2_sem, 16)
            scalar.wait_ge(odma2_sem, 16)

        @block.tensor
        def _(tensor):
            # HAM warmup: garbage matmuls into a scratch bank, no data deps
            for _w in range(NWARM):
                tensor.matmul(
                    ps_warm[0:4, 0:WARM_FD], lhsT=dum_w[:, :], rhs=dum_x[:, :],
                    start=True, stop=True,
                )
            tensor.wait_ge(st_sem, 16)
            waited = set()
            for m in range(NPAIR - 1, -1, -1):
                base = _OFFP[m]
                W = _pair_W(m)
                lhs2 = phist[0:P, m * 2 * RP:(m + 1) * 2 * RP].rearrange(
                    "k (two r) -> k two r", two=2)
                for (c0, c1) in _chunks_of_pair(m):
                    gi = unit_for(base + 2 * c0)
                    if gi not in waited:
                        waited.add(gi)
                        tensor.wait_ge(u_sems[gi], 16)
                    w = c1 - c0
                    if USE_DR and m in DR_PAIRS:
                        rhs2 = mt[0:P, base + 2 * c0:base + 2 * c1].rearrange(
                            "k (two w) -> k two w", two=2)
                        tensor.matmul(
                            ps[0:RP, c0:c1], lhsT=lhs2, rhs=rhs2,
                            start=(m == NPAIR - 1 and c0 % CHUNK == 0),
                            stop=(c0 // 256 == m),
                            perf_mode=mybir.MatmulPerfMode.DoubleRow,
                        ).then_inc(pe_sem, 1)
                    else:
                        tensor.matmul(
                            ps[0:RP, c0:c1],
                            lhsT=phist[0:P, m * 2 * RP:m * 2 * RP + RP],
                            rhs=mt[0:P, base + 2 * c0:base + 2 * c0 + w],
                            start=(m == NPAIR - 1 and c0 % CHUNK == 0), stop=False,
                        )
                        tensor.matmul(
                            ps[0:RP, c0:c1],
                            lhsT=phist[0:P, m * 2 * RP + RP:(m + 1) * 2 * RP],
                            rhs=mt[0:P, base + 2 * c0 + w:base + 2 * c1],
                            start=False,
                            stop=(c0 // 256 == m),
                        ).then_inc(pe_sem, 1)
                # strip: odd block's last 128 cols, plain fp8 matmul
                gi = unit_for(base + 2 * W)
                if gi not in waited:
                    waited.add(gi)
                    tensor.wait_ge(u_sems[gi], 16)
                tensor.matmul(
                    ps[0:RP, W:W + P],
                    lhsT=phist[0:P, m * 2 * RP:m * 2 * RP + RP],
                    rhs=mt[0:P, base + 2 * W:base + 2 * W + P],
                    start=(m == NPAIR - 1 and W % CHUNK == 0),
                    stop=True,
                ).then_inc(pe_sem, 1)

        @block.vector
        def _(vector):
            vector.wait_ge(pt_sem, 16)
            for si, (c0, c1) in enumerate(DVE_SPANS):
                vector.wait_ge(pe_sem, dve_thr[si])
                w = c1 - c0
                # single-pass fold: out=(ps*1.0)*phit, accum_out=row-sum
                # (tensor_tensor_reduce crashes the exec unit on this runtime)
                vector.scalar_tensor_tensor(
                    out=prod[0:RP, 0:w],
                    in0=ps[0:RP, c0:c1],
                    scalar=1.0,
                    in1=phit[0:RP, c0:c1],
                    op0=mybir.AluOpType.mult,
                    op1=mybir.AluOpType.mult,
                    accum_out=acc[0:RP, si:si + 1],
                ).then_inc(dve_sem, 1)

    nc.compile()
    return nc


_FEATS = [(k1, k2) for k1 in range(KDEG + 1) for k2 in range(KDEG + 1 - k1)]


def _features(pos):
    """pos [N, 2] f64 -> Phi [N, R] f64."""
    x, y = pos[:, 0], pos[:, 1]
    base = np.exp(-(x * x + y * y) / TEMPERATURE)
    cols = [
        base * (x / math.sqrt(5.0)) ** k1 * (y / math.sqrt(5.0)) ** k2
        / math.sqrt(math.factorial(k1) * math.factorial(k2))
        for (k1, k2) in _FEATS
    ]
    return np.stack(cols, axis=1)


def _host_prep(embedding, abs_coords, patch_mask):
    in_maps = []
    count1 = 0
    diag_cnt = 0
    for b in range(B):
        pos = embedding[b].astype(np.float64) + abs_coords[b].astype(np.float64)
        Phi = _features(pos)                                        # [N, R]
        alpha = np.exp2(np.round(np.log2(160.0 / np.abs(Phi).max(axis=0))))
        Phi8 = np.clip(Phi * alpha, -240, 240).astype(float8_e4m3)  # [N, R]
        PhiT = (Phi / alpha).T.astype(np.float16)                   # [R, N]

        phist = np.zeros((P, NPAIR * 2 * RP), dtype=float8_e4m3)
        for m in range(NPAIR):
            J1, J0 = 2 * m + 1, 2 * m
            phist[:, m * 2 * RP:m * 2 * RP + R] = Phi8[J1 * P:(J1 + 1) * P, :]
            phist[:, m * 2 * RP + RP:m * 2 * RP + RP + R] = Phi8[J0 * P:(J0 + 1) * P, :]
        phit = np.zeros((RP, N), dtype=np.float16)
        phit[0:R, :] = PhiT

        m_ = patch_mask[b] == 1
        count1 += int(m_.sum())
        diag_cnt += int(np.trace(m_))
        msum = m_.astype(np.int8) + m_.astype(np.int8).T
        Mt8 = (np.triu(msum, k=1).astype(np.float32) * 0.5).astype(float8_e4m3)
        mt = np.zeros((P, MTW), dtype=float8_e4m3)
        for m in range(NPAIR):
            base = _OFFP[m]
            W = _pair_W(m)
            J1, J0 = 2 * m + 1, 2 * m
            for (c0, c1) in _chunks_of_pair(m):
                w = c1 - c0
                mt[:, base + 2 * c0:base + 2 * c0 + w] = \
                    Mt8[c0:c1, J1 * P:(J1 + 1) * P].T
                mt[:, base + 2 * c0 + w:base + 2 * c1] = \
                    Mt8[c0:c1, J0 * P:(J0 + 1) * P].T
            mt[:, base + 2 * W:base + 2 * W + P] = \
                Mt8[W:W + P, J1 * P:(J1 + 1) * P].T
        in_maps.append({"phist": phist, "phit": phit, "mt": mt})
    return in_maps, count1, diag_cnt


def kernel(embedding, abs_coords, patch_mask):
    global LAST_RESULTS
    from concourse.bass_utils import run_bass_kernel_spmd

    embedding = np.asarray(embedding)
    abs_coords = np.asarray(abs_coords)
    patch_mask = np.asarray(patch_mask)

    if "nc" not in _cache:
        _cache["nc"] = _build()
    nc = _cache["nc"]

    in_maps, count1, diag_cnt = _host_prep(embedding, abs_coords, patch_mask)

    res = run_bass_kernel_spmd(
        nc, in_maps, core_ids=list(range(B)),
        trace=TRACE, trace_cores=[0] if TRACE else None,
    )
    LAST_RESULTS = res

    s_hw = sum(res.results[b]["out"].astype(np.float64).sum() for b in range(B))
    loss = np.float64(count1) - np.float64(diag_cnt) - 2.0 * s_hw
    return np.array(loss, dtype=np.float32)


# revision 47
# speedup vs baseline: 1.0911x; 1.0065x over previous
"""Distributed Trainium2 (Bass) kernel for nn_AnchorLoss — rank-R feature path.

Reference:
  pos  = embedding + abs_coords                     [B, N, D],  B=8, N=2048, D=2
  K_ij = exp(-||pos_i - pos_j||^2 / T)
  loss = sum over (b,i,j) with patch_mask==1 of (1 - K_ij)

Math: the Gaussian kernel over ~N(0,2) 2-D data is smooth, so it admits a
low-rank Mercer/Taylor expansion
  K(u,v) = e^{-r_u/T} e^{-r_v/T} e^{u.v/5}
         ~= sum_f Phi_f(u) Phi_f(v),
  Phi_{k1,k2}(u) = e^{-r_u/T} (x/sqrt5)^{k1} (y/sqrt5)^{k2} / sqrt(k1! k2!)
truncated at total degree KDEG=6 (R=28 features, padded to RP=32; measured
end-to-end rel err ~7e-5, gate is 2e-2). With M~ = upper-tri((mask+mask^T)/2,
diag=0):
  loss = count1 - diag_cnt - 2*S,   S = trace(Phi^T M~ Phi)
so the whole masked pairwise sum becomes TensorE matmuls — ZERO on-chip exp
(the baseline's ScalarE exp stream was the measured bottleneck at ~21us).

Distribution: batch b -> NeuronCore b (8 cores). Host combines scalars.

Kernel (per core):
  Psi^T[f, i] = sum_j Phi8[j, f] * Mt[j, i]  accumulated in PSUM f32 over the
  16 column-blocks of the triangle Mt = M~^T (block J = rows j in
  [128J, 128J+128) x cols i in [0, 128(J+1)), fp8 e4m3 — mask values
  {0, 0.5, 1} are fp8-exact; Phi is fp8 with power-of-2 per-feature scales,
  un-scaled on the fp16 PhiT side). Adjacent blocks are PAIRED (15,14)...
  (1,0) sharing one region; each 512-col chunk runs as two plain fp8
  matmuls (sub0/sub1) plus a 128-col strip for the odd block. (A dual-fp8
  DoubleRow variant exists behind USE_DR but is disabled: it intermittently
  crashes the exec unit and buys nothing — the stream is DMA-bound.)
  PSUM start_tensor_calc marks the whole 2KB bank pending-zero, so ONLY the
  bank-aligned first-touch matmul of each bank sets start=True; later
  first-writes inherit the pending flag and overwrite, accumulators add.
  Pairs run DESCENDING so high PSUM cols finalize first; the DVE folds each
  finalized span against PhiT via scalar_tensor_tensor (one op: mult +
  row-sum accumulate). A burst of NWARM dummy matmuls at block start keeps
  the PE busy so the HAM clock gate lifts 1.2->2.4 GHz before real work.
  DMA: the 2.2MB fp8 triangle streams as 8 wide units (one per pair, 128
  descriptors of up to 4KB each) in strict consumption order, alternating
  between the sync and scalar HWDGE queues. Wide units matter: the DMA head
  is descriptor-rate bound (~140-400ns/descriptor for the first ~400
  descriptors regardless of size), so maximizing bytes/descriptor moves
  ~4x the data through the slow head window and leaves no mid-stream
  stalls; finer unit splits measurably regress. Within a pair region the
  two sub-blocks are interleaved at 512-col chunk granularity so each
  chunk's operands arrive together.
"""

from contextlib import ExitStack

import math
import numpy as np
from ml_dtypes import float8_e4m3

B, N, D = 8, 2048, 2
TEMPERATURE = 10.0
P = 128
KDEG = 6
R = (KDEG + 1) * (KDEG + 2) // 2          # 28 real features
RP = 32                                   # padded (DoubleRow needs 16B-aligned steps)
NPAIR = 8                                 # pairs (15,14) ... (1,0)
CHUNK = 512                               # PSUM bank width in f32
NWARM = 46                                # dummy matmuls to un-throttle the PE HAM early
DR_PAIRS = set()                            # pairs using DoubleRow
USE_DR = False                            # DoubleRow dual-fp8 perf mode
WARM_FD = 128                             # fat enough that the HAM sees real PE activity

TRACE = False        # set True (see test.py) to neuron-profile the run
LAST_RESULTS = None  # BassKernelResults of the last run when TRACE

_cache = {}


def _pair_W(m):
    """Shared (paired) column count of pair m = blocks (2m+1, 2m)."""
    return (2 * m + 1) * P


def _chunks_of_pair(m):
    """Chunk list over the paired span [0, W): 512-grid, but the final 128
    cols [W-128, W) form their own chunk — they are the only paired cols
    whose LAST writer is this pair, and stop_tensor_calc (which gates PSUM
    read visibility on hardware) must be exact per element."""
    W = _pair_W(m)
    bounds = list(range(0, W - P, CHUNK)) + [W - P, W]
    return list(zip(bounds[:-1], bounds[1:]))


# region layout (consumption order, pairs descending m=7..0):
#   pair m region = [sub0 c | sub1 c]-interleaved 512-chunks + 128-col strip
_OFFP = {}
_off = 0
for _m in range(NPAIR - 1, -1, -1):
    _OFFP[_m] = _off
    _off += 2 * _pair_W(_m) + P
MTW = _off                                # 17408

# DMA units: ONE per pair, in consumption order. The DMA head is
# descriptor-rate bound (~140-400ns/descriptor for the first ~400
# descriptors regardless of size), so wide units (128 descriptors of up to
# 4KB each) move ~4x the bytes through the slow head window and leave no
# mid-stream stalls.
UNITS = []
for _m in range(NPAIR - 1, -1, -1):
    UNITS.append((_OFFP[_m], _OFFP[_m] + 2 * _pair_W(_m) + P))
SYNC_IDX = list(range(0, len(UNITS), 2))
SCAL_IDX = list(range(1, len(UNITS), 2))

# DVE fold spans (c0, c1) -> last pair touching span is m = c0 // 256
DVE_SPANS = [(1536, 2048), (1024, 1536), (512, 1024), (256, 512), (0, 256)]


def _build():
    from concourse import bacc, mybir

    nc = bacc.Bacc(enable_partition_id=False)
    f32 = mybir.dt.float32
    f16 = mybir.dt.float16
    f8 = mybir.dt.float8e4

    phist_d = nc.declare_dram_parameter("phist", [P, NPAIR * 2 * RP], f8, isOutput=False)
    phit_d = nc.declare_dram_parameter("phit", [RP, N], f16, isOutput=False)
    mt_d = nc.declare_dram_parameter("mt", [P, MTW], f8, isOutput=False)
    out_d = nc.declare_dram_parameter("out", [RP, len(DVE_SPANS)], f32, isOutput=True)

    def unit_for(rc0):
        for gi, (u0, u1) in enumerate(UNITS):
            if u0 <= rc0 < u1:
                return gi
        raise AssertionError(rc0)

    # pe_sem value after the last chunk of pair m (pairs run descending)
    done_after = {}
    cnt = 0
    for m in range(NPAIR - 1, -1, -1):
        cnt += len(_chunks_of_pair(m)) + 1        # + strip
        done_after[m] = cnt
    dve_thr = [done_after[c0 // 256] for (c0, c1) in DVE_SPANS]

    with ExitStack() as ctx:
        phist = ctx.enter_context(nc.sbuf_tensor("phist_sb", [P, NPAIR * 2 * RP], f8))
        phit = ctx.enter_context(nc.sbuf_tensor("phit_sb", [RP, N], f16))
        mt = ctx.enter_context(nc.sbuf_tensor("mt_sb", [P, MTW], f8))
        acc = ctx.enter_context(nc.sbuf_tensor("acc", [RP, len(DVE_SPANS)], f32))
        prod = ctx.enter_context(nc.sbuf_tensor("prod", [RP, CHUNK], f32))
        dum_w = ctx.enter_context(nc.sbuf_tensor("dum_w", [P, 4], f16))
        dum_x = ctx.enter_context(nc.sbuf_tensor("dum_x", [P, WARM_FD], f8))
        ps = ctx.enter_context(nc.psum_tensor("ps", [P, N], f32))
        ps_warm = ctx.enter_context(nc.psum_tensor("ps_warm", [P, WARM_FD], f32))

        u_sems = [ctx.enter_context(nc.semaphore(f"u{g}")) for g in range(len(UNITS))]
        st_sem = ctx.enter_context(nc.semaphore("st"))
        pt_sem = ctx.enter_context(nc.semaphore("pt"))
        pe_sem = ctx.enter_context(nc.semaphore("pe"))
        dve_sem = ctx.enter_context(nc.semaphore("dve"))
        odma_sem = ctx.enter_context(nc.semaphore("odma"))
        odma2_sem = ctx.enter_context(nc.semaphore("odma2"))
        block = ctx.enter_context(nc.Block(no_gpsimd_drain=True))

        @block.sync
        def _(sync):
            sync.dma_start(out=phist[:, :], in_=phist_d[:, :]).then_inc(st_sem, 16)
            for gi in SYNC_IDX:
                c0, c1 = UNITS[gi]
                sync.dma_start(
                    out=mt[0:P, c0:c1], in_=mt_d[0:P, c0:c1]
                ).then_inc(u_sems[gi], 16)
            # output slots 0-3 as soon as their folds are done; the gen and
            # completion latency overlap the final fold
            nsp = len(DVE_SPANS)
            sync.wait_ge(dve_sem, nsp - 1)
            sync.dma_start(out=out_d[:, 0:nsp - 1], in_=acc[:, 0:nsp - 1]).then_inc(odma_sem, 16)
            sync.wait_ge(odma_sem, 16)

        @block.scalar
        def _(scalar):
            for gi in SCAL_IDX:
                c0, c1 = UNITS[gi]
                scalar.dma_start(
                    out=mt[0:P, c0:c1], in_=mt_d[0:P, c0:c1]
                ).then_inc(u_sems[gi], 16)
                if gi == 5:
                    # phit needed by the first DVE fold (~1/3 into the stream)
                    scalar.dma_start(out=phit[:, :], in_=phit_d[:, :]).then_inc(pt_sem, 16)
            # final fold's slot from this queue, in parallel with sync's
            nsp = len(DVE_SPANS)
            scalar.wait_ge(dve_sem, nsp)
            with nc.allow_non_contiguous_dma(reason="32x4B final slot"):
                scalar.dma_start(out=out_d[:, nsp - 1:nsp], in_=acc[:, nsp - 1:nsp]).then_inc(odma2_sem, 16)
            scalar.wait_ge(odma2_sem, 16)

        @block.tensor
        def _(tensor):
            # HAM warmup: garbage matmuls into a scratch bank, no data deps
            for _w in range(NWARM):
                tensor.matmul(
                    ps_warm[0:4, 0:WARM_FD], lhsT=dum_w[:, :], rhs=dum_x[:, :],
                    start=True, stop=True,
                )
            tensor.wait_ge(st_sem, 16)
            waited = set()
            for m in range(NPAIR - 1, -1, -1):
                base = _OFFP[m]
                W = _pair_W(m)
                lhs2 = phist[0:P, m * 2 * RP:(m + 1) * 2 * RP].rearrange(
                    "k (two r) -> k two r", two=2)
                for (c0, c1) in _chunks_of_pair(m):
                    gi = unit_for(base + 2 * c0)
                    if gi not in waited:
                        waited.add(gi)
                        tensor.wait_ge(u_sems[gi], 16)
                    w = c1 - c0
                    if USE_DR and m in DR_PAIRS:
                        rhs2 = mt[0:P, base + 2 * c0:base + 2 * c1].rearrange(
                            "k (two w) -> k two w", two=2)
                        tensor.matmul(
                            ps[0:RP, c0:c1], lhsT=lhs2, rhs=rhs2,
                            start=(m == NPAIR - 1 and c0 % CHUNK == 0),
                            stop=(c0 // 256 == m),
                            perf_mode=mybir.MatmulPerfMode.DoubleRow,
                        ).then_inc(pe_sem, 1)
                    else:
                        tensor.matmul(
                            ps[0:RP, c0:c1],
                            lhsT=phist[0:P, m * 2 * RP:m * 2 * RP + RP],
                            rhs=mt[0:P, base + 2 * c0:base + 2 * c0 + w],
                            start=(m == NPAIR - 1 and c0 % CHUNK == 0), stop=False,
                        )
                        tensor.matmul(
                            ps[0:RP, c0:c1],
                            lhsT=phist[0:P, m * 2 * RP + RP:(m + 1) * 2 * RP],
                            rhs=mt[0:P, base + 2 * c0 + w:base + 2 * c1],
                            start=False,
                            stop=(c0 // 256 == m),
                        ).then_inc(pe_sem, 1)
                # strip: odd block's last 128 cols, plain fp8 matmul
                gi = unit_for(base + 2 * W)
                if gi not in waited:
                    waited.add(gi)
                    tensor.wait_ge(u_sems[gi], 16)
                tensor.matmul(
                    ps[0:RP, W:W + P],
                    lhsT=phist[0:P, m * 2 * RP:m * 2 * RP + RP],
                    rhs=mt[0:P, base + 2 * W:base + 2 * W + P],
                    start=(m == NPAIR - 1 and W % CHUNK == 0),
                    stop=True,
                ).then_inc(pe_sem, 1)

        @block.vector
        def _(vector):
            vector.wait_ge(pt_sem, 16)
            for si, (c0, c1) in enumerate(DVE_SPANS):
                vector.wait_ge(pe_sem, dve_thr[si])
                w = c1 - c0
                # single-pass fold: out=(ps*1.0)*phit, accum_out=row-sum
                # (tensor_tensor_reduce crashes the exec unit on this runtime)
                vector.scalar_tensor_tensor(
                    out=prod[0:RP, 0:w],
                    in0=ps[0:RP, c0:c1],
                    scalar=1.0,
                    in1=phit[0:RP, c0:c1],
                    op0=mybir.AluOpType.mult,
                    op1=mybir.AluOpType.mult,
                    accum_out=acc[0:RP, si:si + 1],
                ).then_inc(dve_sem, 1)

    nc.compile()
    return nc


_FEATS = [(k1, k2) for k1 in range(KDEG + 1) for k2 in range(KDEG + 1 - k1)]


def _features(pos):
    """pos [N, 2] f64 -> Phi [N, R] f64."""
    x, y = pos[:, 0], pos[:, 1]
    base = np.exp(-(x * x + y * y) / TEMPERATURE)
    cols = [
        base * (x / math.sqrt(5.0)) ** k1 * (y / math.sqrt(5.0)) ** k2
        / math.sqrt(math.factorial(k1) * math.factorial(k2))
        for (k1, k2) in _FEATS
    ]
    return np.stack(cols, axis=1)


def _host_prep(embedding, abs_coords, patch_mask):
    in_maps = []
    count1 = 0
    diag_cnt = 0
    for b in range(B):
        pos = embedding[b].astype(np.float64) + abs_coords[b].astype(np.float64)
        Phi = _features(pos)                                        # [N, R]
        alpha = np.exp2(np.round(np.log2(160.0 / np.abs(Phi).max(axis=0))))
        Phi8 = np.clip(Phi * alpha, -240, 240).astype(float8_e4m3)  # [N, R]
        PhiT = (Phi / alpha).T.astype(np.float16)                   # [R, N]

        phist = np.zeros((P, NPAIR * 2 * RP), dtype=float8_e4m3)
        for m in range(NPAIR):
            J1, J0 = 2 * m + 1, 2 * m
            phist[:, m * 2 * RP:m * 2 * RP + R] = Phi8[J1 * P:(J1 + 1) * P, :]
            phist[:, m * 2 * RP + RP:m * 2 * RP + RP + R] = Phi8[J0 * P:(J0 + 1) * P, :]
        phit = np.zeros((RP, N), dtype=np.float16)
        phit[0:R, :] = PhiT

        m_ = patch_mask[b] == 1
        count1 += int(m_.sum())
        diag_cnt += int(np.trace(m_))
        msum = m_.astype(np.int8) + m_.astype(np.int8).T
        Mt8 = (np.triu(msum, k=1).astype(np.float32) * 0.5).astype(float8_e4m3)
        mt = np.zeros((P, MTW), dtype=float8_e4m3)
        for m in range(NPAIR):
            base = _OFFP[m]
            W = _pair_W(m)
            J1, J0 = 2 * m + 1, 2 * m
            for (c0, c1) in _chunks_of_pair(m):
                w = c1 - c0
                mt[:, base + 2 * c0:base + 2 * c0 + w] = \
                    Mt8[c0:c1, J1 * P:(J1 + 1) * P].T
                mt[:, base + 2 * c0 + w:base + 2 * c1] = \
                    Mt8[c0:c1, J0 * P:(J0 + 1) * P].T
            mt[:, base + 2 * W:base + 2 * W + P] = \
                Mt8[W:W + P, J1 * P:(J1 + 1) * P].T
        in_maps.append({"phist": phist, "phit": phit, "mt": mt})
    return in_maps, count1, diag_cnt


def kernel(embedding, abs_coords, patch_mask):
    global LAST_RESULTS
    from concourse.bass_utils import run_bass_kernel_spmd

    embedding = np.asarray(embedding)
    abs_coords = np.asarray(abs_coords)
    patch_mask = np.asarray(patch_mask)

    if "nc" not in _cache:
        _cache["nc"] = _build()
    nc = _cache["nc"]

    in_maps, count1, diag_cnt = _host_prep(embedding, abs_coords, patch_mask)

    res = run_bass_kernel_spmd(
        nc, in_maps, core_ids=list(range(B)),
        trace=TRACE, trace_cores=[0] if TRACE else None,
    )
    LAST_RESULTS = res

    s_hw = sum(res.results[b]["out"].astype(np.float64).sum() for b in range(B))
    loss = np.float64(count1) - np.float64(diag_cnt) - 2.0 * s_hw
    return np.array(loss, dtype=np.float32)


# revision 48
# speedup vs baseline: 1.1120x; 1.0192x over previous
"""Distributed Trainium2 (Bass) kernel for nn_AnchorLoss — rank-R feature path.

Reference:
  pos  = embedding + abs_coords                     [B, N, D],  B=8, N=2048, D=2
  K_ij = exp(-||pos_i - pos_j||^2 / T)
  loss = sum over (b,i,j) with patch_mask==1 of (1 - K_ij)

Math: the Gaussian kernel over ~N(0,2) 2-D data is smooth, so it admits a
low-rank Mercer/Taylor expansion
  K(u,v) = e^{-r_u/T} e^{-r_v/T} e^{u.v/5}
         ~= sum_f Phi_f(u) Phi_f(v),
  Phi_{k1,k2}(u) = e^{-r_u/T} (x/sqrt5)^{k1} (y/sqrt5)^{k2} / sqrt(k1! k2!)
truncated at total degree KDEG=6 (R=28 features, padded to RP=32; measured
end-to-end rel err ~7e-5, gate is 2e-2). With M~ = upper-tri((mask+mask^T)/2,
diag=0):
  loss = count1 - diag_cnt - 2*S,   S = trace(Phi^T M~ Phi)
so the whole masked pairwise sum becomes TensorE matmuls — ZERO on-chip exp
(the baseline's ScalarE exp stream was the measured bottleneck at ~21us).

Distribution: batch b -> NeuronCore b (8 cores). Host combines scalars.

Kernel (per core):
  Psi^T[f, i] = sum_j Phi8[j, f] * Mt[j, i]  accumulated in PSUM f32 over the
  16 column-blocks of the triangle Mt = M~^T (block J = rows j in
  [128J, 128J+128) x cols i in [0, 128(J+1)), fp8 e4m3 — mask values
  {0, 0.5, 1} are fp8-exact; Phi is fp8 with power-of-2 per-feature scales,
  un-scaled on the fp16 PhiT side). Adjacent blocks are PAIRED (15,14)...
  (1,0) sharing one region; each 512-col chunk runs as two plain fp8
  matmuls (sub0/sub1) plus a 128-col strip for the odd block. (A dual-fp8
  DoubleRow variant exists behind USE_DR but is disabled: it intermittently
  crashes the exec unit and buys nothing — the stream is DMA-bound.)
  PSUM start_tensor_calc marks the whole 2KB bank pending-zero, so ONLY the
  bank-aligned first-touch matmul of each bank sets start=True; later
  first-writes inherit the pending flag and overwrite, accumulators add.
  Pairs run DESCENDING so high PSUM cols finalize first; the DVE folds each
  finalized span against PhiT via scalar_tensor_tensor (one op: mult +
  row-sum accumulate). A burst of NWARM dummy matmuls at block start keeps
  the PE busy so the HAM clock gate lifts 1.2->2.4 GHz before real work.
  DMA: the 2.2MB fp8 triangle streams as 8 wide units (one per pair, 128
  descriptors of up to 4KB each) in strict consumption order, alternating
  between the sync and scalar HWDGE queues. Wide units matter: the DMA head
  is descriptor-rate bound (~140-400ns/descriptor for the first ~400
  descriptors regardless of size), so maximizing bytes/descriptor moves
  ~4x the data through the slow head window and leaves no mid-stream
  stalls; finer unit splits measurably regress. Within a pair region the
  two sub-blocks are interleaved at 512-col chunk granularity so each
  chunk's operands arrive together.
"""

from contextlib import ExitStack

import math
import numpy as np
from ml_dtypes import float8_e4m3

B, N, D = 8, 2048, 2
TEMPERATURE = 10.0
P = 128
KDEG = 6
R = (KDEG + 1) * (KDEG + 2) // 2          # 28 real features
RP = 32                                   # padded (DoubleRow needs 16B-aligned steps)
NPAIR = 8                                 # pairs (15,14) ... (1,0)
CHUNK = 512                               # PSUM bank width in f32
NWARM = 46                                # dummy matmuls to un-throttle the PE HAM early
DR_PAIRS = set()                            # pairs using DoubleRow
USE_DR = False                            # DoubleRow dual-fp8 perf mode
WARM_FD = 128                             # fat enough that the HAM sees real PE activity

TRACE = False        # set True (see test.py) to neuron-profile the run
LAST_RESULTS = None  # BassKernelResults of the last run when TRACE

_cache = {}


def _pair_W(m):
    """Shared (paired) column count of pair m = blocks (2m+1, 2m)."""
    return (2 * m + 1) * P


def _chunks_of_pair(m):
    """Chunk list over the paired span [0, W): 512-grid, but the final 128
    cols [W-128, W) form their own chunk — they are the only paired cols
    whose LAST writer is this pair, and stop_tensor_calc (which gates PSUM
    read visibility on hardware) must be exact per element."""
    W = _pair_W(m)
    bounds = list(range(0, W - P, CHUNK)) + [W - P, W]
    return list(zip(bounds[:-1], bounds[1:]))


# region layout (consumption order, pairs descending m=7..0):
#   cols [0, PH) hold the phist stationaries (fused into unit 0 so the head
#   carries no separate small-descriptor DMA); pair m region follows as
#   [sub0 c | sub1 c]-interleaved 512-chunks + 128-col strip
PH = NPAIR * 2 * RP                       # 512 stationary cols
_OFFP = {}
_off = PH
for _m in range(NPAIR - 1, -1, -1):
    _OFFP[_m] = _off
    _off += 2 * _pair_W(_m) + P
MTW = _off                                # 512 + 17408

# DMA units: ONE per pair, in consumption order. The DMA head is
# descriptor-rate bound (~140-400ns/descriptor for the first ~400
# descriptors regardless of size), so wide units (128 descriptors of up to
# 4KB each) move ~4x the bytes through the slow head window and leave no
# mid-stream stalls.
UNITS = []
for _m in range(NPAIR - 1, -1, -1):
    _lo = 0 if _m == NPAIR - 1 else _OFFP[_m]
    UNITS.append((_lo, _OFFP[_m] + 2 * _pair_W(_m) + P))
SYNC_IDX = list(range(0, len(UNITS), 2))
SCAL_IDX = list(range(1, len(UNITS), 2))

# DVE fold spans (c0, c1) -> last pair touching span is m = c0 // 256
DVE_SPANS = [(1536, 2048), (1024, 1536), (512, 1024), (256, 512), (0, 256)]


def _build():
    from concourse import bacc, mybir

    nc = bacc.Bacc(enable_partition_id=False)
    f32 = mybir.dt.float32
    f16 = mybir.dt.float16
    f8 = mybir.dt.float8e4

    phit_d = nc.declare_dram_parameter("phit", [RP, N], f16, isOutput=False)
    mt_d = nc.declare_dram_parameter("mt", [P, MTW], f8, isOutput=False)
    out_d = nc.declare_dram_parameter("out", [RP, len(DVE_SPANS)], f32, isOutput=True)

    def unit_for(rc0):
        for gi, (u0, u1) in enumerate(UNITS):
            if u0 <= rc0 < u1:
                return gi
        raise AssertionError(rc0)

    # pe_sem value after the last chunk of pair m (pairs run descending)
    done_after = {}
    cnt = 0
    for m in range(NPAIR - 1, -1, -1):
        cnt += len(_chunks_of_pair(m)) + 1        # + strip
        done_after[m] = cnt
    dve_thr = [done_after[c0 // 256] for (c0, c1) in DVE_SPANS]

    with ExitStack() as ctx:
        phit = ctx.enter_context(nc.sbuf_tensor("phit_sb", [RP, N], f16))
        mt = ctx.enter_context(nc.sbuf_tensor("mt_sb", [P, MTW], f8))
        acc = ctx.enter_context(nc.sbuf_tensor("acc", [RP, len(DVE_SPANS)], f32))
        prod = ctx.enter_context(nc.sbuf_tensor("prod", [RP, CHUNK], f32))
        dum_w = ctx.enter_context(nc.sbuf_tensor("dum_w", [P, 4], f16))
        dum_x = ctx.enter_context(nc.sbuf_tensor("dum_x", [P, WARM_FD], f8))
        ps = ctx.enter_context(nc.psum_tensor("ps", [P, N], f32))
        ps_warm = ctx.enter_context(nc.psum_tensor("ps_warm", [P, WARM_FD], f32))

        u_sems = [ctx.enter_context(nc.semaphore(f"u{g}")) for g in range(len(UNITS))]
        pt_sem = ctx.enter_context(nc.semaphore("pt"))
        pe_sem = ctx.enter_context(nc.semaphore("pe"))
        dve_sem = ctx.enter_context(nc.semaphore("dve"))
        odma_sem = ctx.enter_context(nc.semaphore("odma"))
        odma2_sem = ctx.enter_context(nc.semaphore("odma2"))
        block = ctx.enter_context(nc.Block(no_gpsimd_drain=True))

        @block.sync
        def _(sync):
            for gi in SYNC_IDX:
                c0, c1 = UNITS[gi]
                sync.dma_start(
                    out=mt[0:P, c0:c1], in_=mt_d[0:P, c0:c1]
                ).then_inc(u_sems[gi], 16)
            # output slots 0-3 as soon as their folds are done; the gen and
            # completion latency overlap the final fold
            nsp = len(DVE_SPANS)
            sync.wait_ge(dve_sem, nsp - 1)
            sync.dma_start(out=out_d[:, 0:nsp - 1], in_=acc[:, 0:nsp - 1]).then_inc(odma_sem, 16)
            sync.wait_ge(odma_sem, 16)

        @block.scalar
        def _(scalar):
            for gi in SCAL_IDX:
                c0, c1 = UNITS[gi]
                scalar.dma_start(
                    out=mt[0:P, c0:c1], in_=mt_d[0:P, c0:c1]
                ).then_inc(u_sems[gi], 16)
                if gi == 5:
                    # phit needed by the first DVE fold (~1/3 into the stream)
                    scalar.dma_start(out=phit[:, :], in_=phit_d[:, :]).then_inc(pt_sem, 16)
            # final fold's slot from this queue, in parallel with sync's
            nsp = len(DVE_SPANS)
            scalar.wait_ge(dve_sem, nsp)
            with nc.allow_non_contiguous_dma(reason="32x4B final slot"):
                scalar.dma_start(out=out_d[:, nsp - 1:nsp], in_=acc[:, nsp - 1:nsp]).then_inc(odma2_sem, 16)
            scalar.wait_ge(odma2_sem, 16)

        @block.tensor
        def _(tensor):
            # HAM warmup: garbage matmuls into a scratch bank, no data deps
            for _w in range(NWARM):
                tensor.matmul(
                    ps_warm[0:4, 0:WARM_FD], lhsT=dum_w[:, :], rhs=dum_x[:, :],
                    start=True, stop=True,
                )
            waited = set()
            for m in range(NPAIR - 1, -1, -1):
                base = _OFFP[m]
                W = _pair_W(m)
                lhs2 = mt[0:P, m * 2 * RP:(m + 1) * 2 * RP].rearrange(
                    "k (two r) -> k two r", two=2)
                for (c0, c1) in _chunks_of_pair(m):
                    gi = unit_for(base + 2 * c0)
                    if gi not in waited:
                        waited.add(gi)
                        tensor.wait_ge(u_sems[gi], 16)
                    w = c1 - c0
                    if USE_DR and m in DR_PAIRS:
                        rhs2 = mt[0:P, base + 2 * c0:base + 2 * c1].rearrange(
                            "k (two w) -> k two w", two=2)
                        tensor.matmul(
                            ps[0:RP, c0:c1], lhsT=lhs2, rhs=rhs2,
                            start=(m == NPAIR - 1 and c0 % CHUNK == 0),
                            stop=(c0 // 256 == m),
                            perf_mode=mybir.MatmulPerfMode.DoubleRow,
                        ).then_inc(pe_sem, 1)
                    else:
                        tensor.matmul(
                            ps[0:RP, c0:c1],
                            lhsT=mt[0:P, m * 2 * RP:m * 2 * RP + RP],
                            rhs=mt[0:P, base + 2 * c0:base + 2 * c0 + w],
                            start=(m == NPAIR - 1 and c0 % CHUNK == 0), stop=False,
                        )
                        tensor.matmul(
                            ps[0:RP, c0:c1],
                            lhsT=mt[0:P, m * 2 * RP + RP:(m + 1) * 2 * RP],
                            rhs=mt[0:P, base + 2 * c0 + w:base + 2 * c1],
                            start=False,
                            stop=(c0 // 256 == m),
                        ).then_inc(pe_sem, 1)
                # strip: odd block's last 128 cols, plain fp8 matmul
                gi = unit_for(base + 2 * W)
                if gi not in waited:
                    waited.add(gi)
                    tensor.wait_ge(u_sems[gi], 16)
                tensor.matmul(
                    ps[0:RP, W:W + P],
                    lhsT=mt[0:P, m * 2 * RP:m * 2 * RP + RP],
                    rhs=mt[0:P, base + 2 * W:base + 2 * W + P],
                    start=(m == NPAIR - 1 and W % CHUNK == 0),
                    stop=True,
                ).then_inc(pe_sem, 1)

        @block.vector
        def _(vector):
            vector.wait_ge(pt_sem, 16)
            for si, (c0, c1) in enumerate(DVE_SPANS):
                vector.wait_ge(pe_sem, dve_thr[si])
                w = c1 - c0
                # single-pass fold: out=(ps*1.0)*phit, accum_out=row-sum
                # (tensor_tensor_reduce crashes the exec unit on this runtime)
                vector.scalar_tensor_tensor(
                    out=prod[0:RP, 0:w],
                    in0=ps[0:RP, c0:c1],
                    scalar=1.0,
                    in1=phit[0:RP, c0:c1],
                    op0=mybir.AluOpType.mult,
                    op1=mybir.AluOpType.mult,
                    accum_out=acc[0:RP, si:si + 1],
                ).then_inc(dve_sem, 1)

    nc.compile()
    return nc


_FEATS = [(k1, k2) for k1 in range(KDEG + 1) for k2 in range(KDEG + 1 - k1)]


def _features(pos):
    """pos [N, 2] f64 -> Phi [N, R] f64."""
    x, y = pos[:, 0], pos[:, 1]
    base = np.exp(-(x * x + y * y) / TEMPERATURE)
    cols = [
        base * (x / math.sqrt(5.0)) ** k1 * (y / math.sqrt(5.0)) ** k2
        / math.sqrt(math.factorial(k1) * math.factorial(k2))
        for (k1, k2) in _FEATS
    ]
    return np.stack(cols, axis=1)


def _host_prep(embedding, abs_coords, patch_mask):
    in_maps = []
    count1 = 0
    diag_cnt = 0
    for b in range(B):
        pos = embedding[b].astype(np.float64) + abs_coords[b].astype(np.float64)
        Phi = _features(pos)                                        # [N, R]
        alpha = np.exp2(np.round(np.log2(160.0 / np.abs(Phi).max(axis=0))))
        Phi8 = np.clip(Phi * alpha, -240, 240).astype(float8_e4m3)  # [N, R]
        PhiT = (Phi / alpha).T.astype(np.float16)                   # [R, N]

        phit = np.zeros((RP, N), dtype=np.float16)
        phit[0:R, :] = PhiT

        m_ = patch_mask[b] == 1
        count1 += int(m_.sum())
        diag_cnt += int(np.trace(m_))
        msum = m_.astype(np.int8) + m_.astype(np.int8).T
        Mt8 = (np.triu(msum, k=1).astype(np.float32) * 0.5).astype(float8_e4m3)
        mt = np.zeros((P, MTW), dtype=float8_e4m3)
        for m in range(NPAIR):
            J1, J0 = 2 * m + 1, 2 * m
            mt[:, m * 2 * RP:m * 2 * RP + R] = Phi8[J1 * P:(J1 + 1) * P, :]
            mt[:, m * 2 * RP + RP:m * 2 * RP + RP + R] = Phi8[J0 * P:(J0 + 1) * P, :]
        for m in range(NPAIR):
            base = _OFFP[m]
            W = _pair_W(m)
            J1, J0 = 2 * m + 1, 2 * m
            for (c0, c1) in _chunks_of_pair(m):
                w = c1 - c0
                mt[:, base + 2 * c0:base + 2 * c0 + w] = \
                    Mt8[c0:c1, J1 * P:(J1 + 1) * P].T
                mt[:, base + 2 * c0 + w:base + 2 * c1] = \
                    Mt8[c0:c1, J0 * P:(J0 + 1) * P].T
            mt[:, base + 2 * W:base + 2 * W + P] = \
                Mt8[W:W + P, J1 * P:(J1 + 1) * P].T
        in_maps.append({"phit": phit, "mt": mt})
    return in_maps, count1, diag_cnt


def kernel(embedding, abs_coords, patch_mask):
    global LAST_RESULTS
    from concourse.bass_utils import run_bass_kernel_spmd

    embedding = np.asarray(embedding)
    abs_coords = np.asarray(abs_coords)
    patch_mask = np.asarray(patch_mask)

    if "nc" not in _cache:
        _cache["nc"] = _build()
    nc = _cache["nc"]

    in_maps, count1, diag_cnt = _host_prep(embedding, abs_coords, patch_mask)

    res = run_bass_kernel_spmd(
        nc, in_maps, core_ids=list(range(B)),
        trace=TRACE, trace_cores=[0] if TRACE else None,
    )
    LAST_RESULTS = res

    s_hw = sum(res.results[b]["out"].astype(np.float64).sum() for b in range(B))
    loss = np.float64(count1) - np.float64(diag_cnt) - 2.0 * s_hw
    return np.array(loss, dtype=np.float32)
